# revision 1
# baseline (speedup 1.0000x reference)
"""BiMamba aggregator on 8 TRN2 NeuronCores.

Sharding: 8 independent shards = batch(4) x direction(fwd/bwd). Each core
runs the full 2-layer Mamba stack + attention pooling for one sequence in
one direction (backward cores get the time-flipped sequence). Host only
flips/concats and applies the final [4,1024] layernorm.

On-core layout: activations are feature-major [feature on partitions,
time on free]. All matmuls run in bf16 (host-precast weights, fp32 PSUM
accumulation), weights in native [in,out] layout so no transposes exist
anywhere. The selective-scan uses the DVE hardware scan instruction
(tensor_tensor_scan) over lanes=(d,s) chunks, with the DS=16 reduction
done by PE identity-matmul accumulation into PSUM.
"""
import numpy as np
import ml_dtypes

import concourse.bass as bass
import concourse.tile as tile
from concourse import mybir
from concourse import bass_utils

F32 = mybir.dt.float32
BF16 = mybir.dt.bfloat16
AF = mybir.ActivationFunctionType
OP = mybir.AluOpType

DM, DI, DS, DC, DTR, L = 512, 1024, 16, 4, 32, 2
Bb, N = 4, 1024
NT2 = N // 2          # 512, matmul moving-dim tile
SBLK = 4              # s-values per scan block (DS/SBLK blocks)

BF = ml_dtypes.bfloat16


# ---------------------------------------------------------------------------
# walrus codegen accepts at most ONE semaphore wait per instruction; Tile can
# emit more. Split the excess onto injected same-engine NoOps.
_EXEMPT = (
    mybir.InstEventSemaphore,
    mybir.InstAllEngineBarrier,
    mybir.InstHalt,
    mybir.InstCall,
)


def _legalize_waits(nc) -> int:
    n_split = 0
    for f in nc.m.functions:
        for bb in f.blocks:
            insts = bb.instructions
            if not any(
                (not isinstance(i, _EXEMPT))
                and i.sync_info is not None
                and len(i.sync_info.on_wait) > 1
                for i in insts
            ):
                continue
            new = []
            for i in insts:
                si = i.sync_info
                if isinstance(i, _EXEMPT) or si is None:
                    new.append(i)
                    continue
                waits = list(si.on_wait)
                if len(waits) <= 1:
                    new.append(i)
                    continue
                for w in waits[:-1]:
                    nop = mybir.InstNoOp(
                        name=f"{i.name}-wsplit{n_split}",
                        engine=i.engine,
                        sync_info=mybir.SyncInfo(on_wait=[w], on_update=[]),
                    )
                    new.append(nop)
                    n_split += 1
                i.sync_info = mybir.SyncInfo(
                    on_wait=waits[-1:], on_update=list(si.on_update)
                )
                new.append(i)
            bb.instructions = new
    return n_split


# ---------------------------------------------------------------------------
def build_nc(debug=False):
    nc = bass.Bass("TRN2", target_bir_lowering=False, debug=False)

    # ---- DRAM I/O (per-core names; same program on all 8 cores) ----
    x_d = nc.dram_tensor("x_d", [DM, N], F32, kind="ExternalInput")
    wt = {}

    def din(name, shape, dt):
        wt[name] = nc.dram_tensor(name, shape, dt, kind="ExternalInput")

    din("inw", [L, DM, 2 * DI], BF16)
    din("cw", [L, DI, DC], F32)
    din("cb", [L, DI], F32)
    din("xpw", [L, DI, DTR + 2 * DS], BF16)
    din("dtw", [L, DTR, DI], BF16)
    din("dtb", [L, DI], F32)
    din("alog", [L, DI, DS], F32)
    din("dd", [L, DI], F32)
    din("ow", [L, DI, DM], BF16)
    din("n1w", [L, DM], F32)
    din("n1b", [L, DM], F32)
    din("n2w", [L, DM], F32)
    din("n2b", [L, DM], F32)
    din("w1", [L, DM, 4 * DM], BF16)
    din("b1", [L, 4 * DM], F32)
    din("w2", [L, 4 * DM, DM], BF16)
    din("b2", [L, DM], F32)
    din("aw1", [DM, DM // 2], BF16)
    din("ab1", [DM // 2], F32)
    din("aw2", [DM // 2, 1], BF16)
    din("ab2", [1], F32)
    din("ones_colT", [128, 1], BF16)   # LN-stats matmul lhsT
    din("ident", [128, 128], BF16)     # scan s-reduction lhsT

    zh_out = nc.dram_tensor("zh", [DM], F32, kind="ExternalOutput")
    av_out = nc.dram_tensor("av", [N], F32, kind="ExternalOutput")
    dbg = {}
    if debug:
        for nm, shape, dt in [
            ("d_xhs0", [DI, N], BF16), ("d_dt0", [DI, N], BF16),
            ("d_y0", [DI, N], F32), ("d_h1", [DM, N], F32),
            ("d_h2", [DM, N], F32), ("d_hf", [DM, N], F32),
        ]:
            dbg[nm] = nc.dram_tensor(nm, shape, dt, kind="ExternalOutput")

    with tile.TileContext(nc) as tc:
        _emit(nc, tc, x_d, wt, zh_out, av_out, dbg)

    n = _legalize_waits(nc)
    return nc


def _emit(nc, tc, x_d, wt, zh_out, av_out, dbg):
    import contextlib
    ctx = contextlib.ExitStack()
    with ctx:
        sb = ctx.enter_context(tc.tile_pool(name="sb", bufs=1))
        ps = ctx.enter_context(tc.tile_pool(name="ps", bufs=1, space="PSUM"))
        dr = ctx.enter_context(tc.tile_pool(name="dr", bufs=1, space="DRAM"))

        def st(shape, dt, tag, bufs):
            return sb.tile(shape, dt, tag=tag, bufs=bufs, name=tag)

        # ---- constants ----
        ones_colT = sb.tile([128, 1], BF16, tag="cones", name="cones")
        nc.sync.dma_start(out=ones_colT, in_=wt["ones_colT"].ap())
        ident = sb.tile([128, 128], BF16, tag="cident", name="cident")
        nc.sync.dma_start(out=ident, in_=wt["ident"].ap())
        eps_t = sb.tile([1, 1], F32, tag="ceps", name="ceps")
        nc.vector.memset(eps_t, 1e-5)

        # ---- load x as h gen-0 (feature-major) ----
        h = []
        for m in range(4):
            t = st([128, N], F32, "h", 4)
            nc.sync.dma_start(out=t, in_=x_d.ap()[m * 128:(m + 1) * 128, :])
            h.append(t)

        # ---- per-(layer,name) packed column constants: [128, count*width],
        # column block m holds values for feature rows m*128..(m+1)*128 ----
        _COLSPEC = {"cw": (8, DC), "cb": (8, 1), "dtb": (8, 1), "dd": (8, 1),
                    "n1w": (4, 1), "n1b": (4, 1), "n2w": (4, 1),
                    "n2b": (4, 1), "b1": (16, 1), "b2": (4, 1)}
        cols = {}

        def col(name, l, m):
            cnt, width = _COLSPEC[name]
            key = (name, l)
            if key not in cols:
                t = sb.tile([128, cnt * width], F32, tag=f"{name}{l}",
                            name=f"{name}{l}")
                src = bass.AP(
                    tensor=wt[name], offset=l * cnt * 128 * width,
                    ap=[[width, 128], [128 * width, cnt], [1, width]])
                dst = t[:].rearrange("p (m k) -> p m k", k=width)
                nc.sync.dma_start(out=dst, in_=src)
                cols[key] = t
            t = cols[key]
            return t[:, m * width:(m + 1) * width]

        def layernorm(l, h_tiles, wname, bname, out_tag):
            """h (4x[128,N] f32) -> normalized bf16 tiles 4x[128,N]."""
            # stats via PE ones-reduce over bf16 casts
            psum_mu = [ps.tile([1, NT2], F32, tag="stat", bufs=4, name="psmu")
                       for _ in range(2)]
            psum_sq = [ps.tile([1, NT2], F32, tag="stat", bufs=4, name="pssq")
                       for _ in range(2)]
            for m in range(4):
                hb = st([128, N], BF16, "u", 2)
                nc.scalar.copy(hb, h_tiles[m])
                sq = st([128, N], BF16, "a", 2)
                nc.scalar.activation(sq, h_tiles[m], AF.Square)
                for n in range(2):
                    nc.tensor.matmul(
                        psum_mu[n], ones_colT,
                        hb[:, n * NT2:(n + 1) * NT2],
                        start=(m == 0), stop=(m == 3))
                    nc.tensor.matmul(
                        psum_sq[n], ones_colT,
                        sq[:, n * NT2:(n + 1) * NT2],
                        start=(m == 0), stop=(m == 3))
            mu = sb.tile([1, N], F32, tag="lnrow", bufs=3, name="mu")
            musq = sb.tile([1, N], F32, tag="lnrow", bufs=3, name="musq")
            var = sb.tile([1, N], F32, tag="lnrow", bufs=3, name="var")
            for n in range(2):
                sl = slice(n * NT2, (n + 1) * NT2)
                nc.vector.tensor_scalar_mul(mu[:, sl], psum_mu[n], 1.0 / DM)
                nc.vector.tensor_mul(musq[:, sl], mu[:, sl], mu[:, sl])
                nc.vector.scalar_tensor_tensor(
                    out=var[:, sl], in0=psum_sq[n], scalar=1.0 / DM,
                    in1=musq[:, sl], op0=OP.mult, op1=OP.subtract)
            sd = sb.tile([1, N], F32, tag="lnrow", bufs=3, name="sd")
            nc.scalar.activation(sd, var, AF.Sqrt, bias=eps_t[:])
            rinv = sb.tile([1, N], F32, tag="lnrow", bufs=3, name="rinv")
            nc.vector.reciprocal(rinv, sd)
            # broadcast mu/rinv across partitions via DRAM bounce
            lnsc = dr.tile([2, N], F32, tag=f"lnsc_{l}_{out_tag}",
                           name=f"lnsc_{l}_{out_tag}")
            nc.sync.dma_start(out=lnsc[0:1, :], in_=mu)
            nc.sync.dma_start(out=lnsc[1:2, :], in_=rinv)
            mb = st([128, N], F32, "f32big", 8)
            nc.sync.dma_start(out=mb, in_=bass.AP(
                tensor=lnsc.tensor, offset=lnsc.offset, ap=[[0, 128], [1, N]]))
            rb = st([128, N], F32, "f32big", 8)
            nc.sync.dma_start(out=rb, in_=bass.AP(
                tensor=lnsc.tensor, offset=lnsc.offset + N,
                ap=[[0, 128], [1, N]]))
            outs = []
            for m in range(4):
                s1 = st([128, N], F32, "f32big", 8)
                nc.gpsimd.tensor_sub(s1, h_tiles[m], mb)
                s2 = st([128, N], F32, "f32big", 8)
                nc.gpsimd.tensor_mul(s2, s1, rb)
                xo = st([128, N], BF16, out_tag, 8)
                nc.vector.tensor_scalar(
                    out=xo, in0=s2, scalar1=col(wname, l, m),
                    scalar2=col(bname, l, m), op0=OP.mult, op1=OP.add)
                outs.append(xo)
            return outs

        # =================== layers ===================
        for l in range(L):
            # ---- LN1 -> xn (bf16, 4 tiles) ----
            xn = layernorm(l, h, "n1w", "n1b", "bfC")

            # ---- in_proj: xz = xn @ inw ; xh raw (padded) + silu(z) ----
            inw_sb = []
            for j in range(4):
                t = st([128, 2 * DI], BF16, "w2048", 4)
                nc.sync.dma_start(
                    out=t, in_=wt["inw"].ap()[l, j * 128:(j + 1) * 128, :])
                inw_sb.append(t)
            xh = [st([128, DC - 1 + N], BF16, "bfF", 8) for _ in range(8)]
            for m in range(8):
                nc.vector.memset(xh[m][:, 0:DC - 1], 0.0)
            # silu(z) stored as 16 half-tiles (slots shared with FFN gf)
            sz = [st([128, NT2], BF16, "bfA", 16) for _ in range(16)]
            for m in range(16):
                for n in range(2):
                    pm = ps.tile([128, NT2], F32, tag="mm", bufs=4, name="pmm")
                    for j in range(4):
                        nc.tensor.matmul(
                            pm, inw_sb[j][:, m * 128:(m + 1) * 128],
                            xn[j][:, n * NT2:(n + 1) * NT2],
                            start=(j == 0), stop=(j == 3))
                    if m < 8:
                        nc.scalar.copy(
                            xh[m][:, DC - 1 + n * NT2:DC - 1 + (n + 1) * NT2],
                            pm)
                    else:
                        nc.scalar.activation(
                            sz[(m - 8) * 2 + n], pm, AF.Silu)

            # ---- causal depthwise conv + silu -> xhs (bf16) ----
            xhs = []
            for m in range(8):
                cwc = col("cw", l, m)
                acc = st([128, N], F32, "f32big", 8)
                nc.vector.tensor_scalar_mul(acc, xh[m][:, 0:N], cwc[:, 0:1])
                for k in range(1, DC):
                    acc2 = st([128, N], F32, "f32big", 8)
                    nc.vector.scalar_tensor_tensor(
                        out=acc2, in0=xh[m][:, k:k + N], scalar=cwc[:, k:k + 1],
                        in1=acc, op0=OP.mult, op1=OP.add)
                    acc = acc2
                t = st([128, N], BF16, "bfE", 8)
                nc.scalar.activation(t, acc, AF.Silu, bias=col("cb", l, m))
                xhs.append(t)

            # ---- x_proj: dbl = xhs @ xpw  ([64, N]) ----
            xpw_sb = []
            for j in range(8):
                t = sb.tile([128, DTR + 2 * DS], BF16, tag=f"xpw{l}_{j}",
                            name=f"xpw{l}_{j}")
                nc.sync.dma_start(
                    out=t, in_=wt["xpw"].ap()[l, j * 128:(j + 1) * 128, :])
                xpw_sb.append(t)
            dbl = sb.tile([64, N], BF16, tag="dblbf", bufs=2, name="dbl")
            for n in range(2):
                pm = ps.tile([64, NT2], F32, tag="mm", bufs=4, name="pdbl")
                for j in range(8):
                    nc.tensor.matmul(pm, xpw_sb[j],
                                     xhs[j][:, n * NT2:(n + 1) * NT2],
                                     start=(j == 0), stop=(j == 7))
                nc.scalar.copy(dbl[:, n * NT2:(n + 1) * NT2], pm)

            # B/C rows -> DRAM for partition-broadcast
            bcsc = dr.tile([2 * DS, N], BF16, tag=f"bcsc{l}", name=f"bcsc{l}")
            nc.sync.dma_start(out=bcsc, in_=dbl[DTR:DTR + 2 * DS, :])

            # ---- dt_proj: dt = softplus(dbl[:,:32] @ dtw + dtb) ----
            # softplus has no ACT table; store dt_neg = ln(sigmoid(-w)) =
            # -softplus(w). The sign is absorbed downstream: decay scale uses
            # +exp(alog), and u/v enter the y accumulation via subtract.
            dtw_sb = sb.tile([DTR, DI], BF16, tag=f"dtw{l}", name=f"dtw{l}")
            nc.sync.dma_start(out=dtw_sb, in_=wt["dtw"].ap()[l])
            col("dtb", l, 0)  # ensure packed tile exists
            ndtb = sb.tile([128, 8], F32, tag="ndtb", bufs=1, name=f"ndtb{l}")
            nc.vector.tensor_scalar_mul(ndtb, cols[("dtb", l)][:], -1.0)
            dt_bf, dtx = [], []
            for m in range(8):
                t = st([128, N], BF16, "bfC", 8)
                for n in range(2):
                    pm = ps.tile([128, NT2], F32, tag="mm", bufs=4, name="pdt")
                    nc.tensor.matmul(pm, dtw_sb[:, m * 128:(m + 1) * 128],
                                     dbl[0:DTR, n * NT2:(n + 1) * NT2],
                                     start=True, stop=True)
                    sg = st([128, NT2], F32, "f32big", 8)
                    nc.scalar.activation(sg, pm, AF.Sigmoid, scale=-1.0,
                                         bias=ndtb[:, m:m + 1])
                    nc.scalar.activation(t[:, n * NT2:(n + 1) * NT2], sg,
                                         AF.Ln)
                dt_bf.append(t)
                tx = st([128, N], BF16, "bfF", 8)
                nc.vector.tensor_mul(tx, t, xhs[m])  # = -dt*xh
                dtx.append(tx)

            # ---- An = +exp(alog) columns (positive |A|) ----
            An = []
            for m in range(8):
                al = sb.tile([128, DS], F32, tag=f"alog{l}_{m}",
                             name=f"alog{l}_{m}")
                nc.sync.dma_start(
                    out=al, in_=wt["alog"].ap()[l, m * 128:(m + 1) * 128, :])
                ea = sb.tile([128, DS], F32, tag=f"An{l}_{m}",
                             name=f"An{l}_{m}")
                nc.scalar.activation(ea, al, AF.Exp)
                An.append(ea)

            # ---- scan stage ----
            y = [st([128, N], F32, "f32big", 8) for _ in range(8)]
            for blk in range(DS // SBLK):
                bbs, cbs = [], []
                for si in range(SBLK):
                    s = blk * SBLK + si
                    bt = st([128, N], BF16, "BC", 8)
                    nc.sync.dma_start(out=bt, in_=bass.AP(
                        tensor=bcsc.tensor, offset=bcsc.offset + s * N,
                        ap=[[0, 128], [1, N]]))
                    bbs.append(bt)
                    ct = st([128, N], BF16, "BC", 8)
                    nc.sync.dma_start(out=ct, in_=bass.AP(
                        tensor=bcsc.tensor, offset=bcsc.offset + (DS + s) * N,
                        ap=[[0, 128], [1, N]]))
                    cbs.append(ct)
                for m in range(8):
                    py = [ps.tile([128, NT2], F32, tag="mm", bufs=4,
                                  name="py") for _ in range(2)]
                    for si in range(SBLK):
                        s = blk * SBLK + si
                        a_s = st([128, N], BF16, "a", 2)
                        nc.scalar.activation(a_s, dt_bf[m], AF.Exp,
                                             scale=An[m][:, s:s + 1])
                        u_s = st([128, N], BF16, "u", 2)
                        ueng = nc.gpsimd if (si % 2 == 1) else nc.vector
                        ueng.tensor_mul(u_s, dtx[m], bbs[si])
                        h_s = st([128, N], BF16, "hh", 2)
                        nc.vector.tensor_tensor_scan(
                            h_s, a_s, u_s, 0.0, OP.mult, OP.add)
                        v_s = st([128, N], BF16, "v", 2)
                        nc.vector.tensor_mul(v_s, h_s, cbs[si])
                        for n in range(2):
                            nc.tensor.matmul(
                                py[n], ident, v_s[:, n * NT2:(n + 1) * NT2],
                                start=(si == 0), stop=(si == SBLK - 1))
                    for n in range(2):
                        # py holds -contribution (u was built from -dt*xh)
                        ysl = y[m][:, n * NT2:(n + 1) * NT2]
                        if blk == 0:
                            nc.vector.scalar_tensor_tensor(
                                out=ysl, in0=xhs[m][:, n * NT2:(n + 1) * NT2],
                                scalar=col("dd", l, m), in1=py[n],
                                op0=OP.mult, op1=OP.subtract)
                        else:
                            nc.vector.tensor_sub(ysl, ysl, py[n])

            # ---- gate with silu(z), out_proj, residual ----
            ow_sb = []
            for j in range(8):
                t = st([128, DM], BF16, "w512", 16)
                nc.sync.dma_start(
                    out=t, in_=wt["ow"].ap()[l, j * 128:(j + 1) * 128, :])
                ow_sb.append(t)
            y3 = []
            for m in range(8):
                t = st([128, N], BF16, "bfC", 8)
                for n in range(2):
                    nc.vector.tensor_mul(t[:, n * NT2:(n + 1) * NT2],
                                         y[m][:, n * NT2:(n + 1) * NT2],
                                         sz[m * 2 + n])
                y3.append(t)
            for mo in range(4):
                for n in range(2):
                    pm = ps.tile([128, NT2], F32, tag="mm", bufs=4, name="pop")
                    for j in range(8):
                        nc.tensor.matmul(
                            pm, ow_sb[j][:, mo * 128:(mo + 1) * 128],
                            y3[j][:, n * NT2:(n + 1) * NT2],
                            start=(j == 0), stop=(j == 7))
                    nc.vector.tensor_add(
                        h[mo][:, n * NT2:(n + 1) * NT2],
                        h[mo][:, n * NT2:(n + 1) * NT2], pm)

            if dbg and l == 0:
                for m in range(8):
                    nc.sync.dma_start(
                        out=dbg["d_xhs0"].ap()[m * 128:(m + 1) * 128, :],
                        in_=xhs[m])
                    nc.sync.dma_start(
                        out=dbg["d_dt0"].ap()[m * 128:(m + 1) * 128, :],
                        in_=dt_bf[m])
                    nc.sync.dma_start(
                        out=dbg["d_y0"].ap()[m * 128:(m + 1) * 128, :],
                        in_=y[m])
                for m in range(4):
                    nc.sync.dma_start(
                        out=dbg["d_h1"].ap()[m * 128:(m + 1) * 128, :],
                        in_=h[m])

            # ---- LN2 + FFN ----
            hn = layernorm(l, h, "n2w", "n2b", "bfC")
            w1_sb = []
            for j in range(4):
                t = st([128, 4 * DM], BF16, "w2048", 4)
                nc.sync.dma_start(
                    out=t, in_=wt["w1"].ap()[l, j * 128:(j + 1) * 128, :])
                w1_sb.append(t)
            w2_sb = []
            for j in range(16):
                t = st([128, DM], BF16, "w512", 16)
                nc.sync.dma_start(
                    out=t, in_=wt["w2"].ap()[l, j * 128:(j + 1) * 128, :])
                w2_sb.append(t)
            # FFN per time-half so only 16 gf tiles are live at once
            for n in range(2):
                gf = [st([128, NT2], BF16, "bfA", 16) for _ in range(16)]
                for m in range(16):
                    pm = ps.tile([128, NT2], F32, tag="mm", bufs=4, name="pw1")
                    for j in range(4):
                        nc.tensor.matmul(
                            pm, w1_sb[j][:, m * 128:(m + 1) * 128],
                            hn[j][:, n * NT2:(n + 1) * NT2],
                            start=(j == 0), stop=(j == 3))
                    nc.scalar.activation(gf[m], pm, AF.Gelu,
                                         bias=col("b1", l, m))
                for mo in range(4):
                    pm = ps.tile([128, NT2], F32, tag="mm", bufs=4, name="pw2")
                    for j in range(16):
                        nc.tensor.matmul(
                            pm, w2_sb[j][:, mo * 128:(mo + 1) * 128],
                            gf[j], start=(j == 0), stop=(j == 15))
                    nc.vector.scalar_tensor_tensor(
                        out=h[mo][:, n * NT2:(n + 1) * NT2], in0=pm,
                        scalar=col("b2", l, mo),
                        in1=h[mo][:, n * NT2:(n + 1) * NT2],
                        op0=OP.add, op1=OP.add)

            if dbg and l == 0:
                for m in range(4):
                    nc.sync.dma_start(
                        out=dbg["d_h2"].ap()[m * 128:(m + 1) * 128, :],
                        in_=h[m])

        # =================== attention pooling ===================
        if dbg:
            for m in range(4):
                nc.sync.dma_start(
                    out=dbg["d_hf"].ap()[m * 128:(m + 1) * 128, :], in_=h[m])
        aw1_sb = []
        for j in range(4):
            t = sb.tile([128, DM // 2], BF16, tag=f"aw1_{j}", name=f"aw1_{j}")
            nc.sync.dma_start(out=t,
                              in_=wt["aw1"].ap()[j * 128:(j + 1) * 128, :])
            aw1_sb.append(t)
        ab1c = []
        for mg in range(2):
            t = sb.tile([128, 1], F32, tag=f"ab1_{mg}", name=f"ab1_{mg}")
            nc.sync.dma_start(
                out=t, in_=wt["ab1"].ap()[mg * 128:(mg + 1) * 128][:, None])
            ab1c.append(t)
        hbf = []
        for m in range(4):
            t = st([128, N], BF16, "BC", 8)
            nc.scalar.copy(t, h[m])
            hbf.append(t)
        g1 = []
        for mg in range(2):
            t = st([128, N], BF16, "bfF", 8)
            for n in range(2):
                pm = ps.tile([128, NT2], F32, tag="mm", bufs=4, name="pg1")
                for j in range(4):
                    nc.tensor.matmul(
                        pm, aw1_sb[j][:, mg * 128:(mg + 1) * 128],
                        hbf[j][:, n * NT2:(n + 1) * NT2],
                        start=(j == 0), stop=(j == 3))
                nc.scalar.activation(t[:, n * NT2:(n + 1) * NT2], pm,
                                     AF.Tanh, bias=ab1c[mg])
            g1.append(t)
        aw2_sb = []
        for mg in range(2):
            t = sb.tile([128, 1], BF16, tag=f"aw2_{mg}", name=f"aw2_{mg}")
            nc.sync.dma_start(out=t,
                              in_=wt["aw2"].ap()[mg * 128:(mg + 1) * 128, :])
            aw2_sb.append(t)
        ab2_sb = sb.tile([1, 1], F32, tag="ab2", name="ab2")
        nc.sync.dma_start(out=ab2_sb, in_=wt["ab2"].ap()[None, :])
        lrow = sb.tile([1, N], F32, tag="lnrow", bufs=3, name="lrow")
        for n in range(2):
            pm = ps.tile([1, NT2], F32, tag="mm", bufs=4, name="pl")
            for mg in range(2):
                nc.tensor.matmul(pm, aw2_sb[mg],
                                 g1[mg][:, n * NT2:(n + 1) * NT2],
                                 start=(mg == 0), stop=(mg == 1))
            nc.vector.tensor_scalar_add(lrow[:, n * NT2:(n + 1) * NT2], pm,
                                        ab2_sb[:])
        mx = sb.tile([1, 1], F32, tag="tiny", bufs=4, name="mx")
        nc.vector.tensor_reduce(mx, lrow, mybir.AxisListType.X, OP.max)
        nmx = sb.tile([1, 1], F32, tag="tiny", bufs=4, name="nmx")
        nc.vector.tensor_scalar_mul(nmx, mx, -1.0)
        erow = sb.tile([1, N], F32, tag="lnrow", bufs=3, name="erow")
        nc.scalar.activation(erow, lrow, AF.Exp, bias=nmx[:])
        ssum = sb.tile([1, 1], F32, tag="tiny", bufs=4, name="ssum")
        nc.vector.tensor_reduce(ssum, erow, mybir.AxisListType.X, OP.add)
        rs = sb.tile([1, 1], F32, tag="tiny", bufs=4, name="rs")
        nc.vector.reciprocal(rs, ssum)
        arow = sb.tile([1, N], F32, tag="lnrow", bufs=3, name="arow")
        nc.vector.tensor_scalar_mul(arow, erow, rs[:])
        nc.sync.dma_start(out=av_out.ap()[None, :], in_=arow)
        # broadcast a over partitions, weighted-sum h over time
        absc = dr.tile([1, N], F32, tag="absc", name="absc")
        nc.sync.dma_start(out=absc, in_=arow)
        ab = st([128, N], F32, "f32big", 8)
        nc.sync.dma_start(out=ab, in_=bass.AP(
            tensor=absc.tensor, offset=absc.offset, ap=[[0, 128], [1, N]]))
        for m in range(4):
            junk = st([128, N], F32, "f32big", 8)
            nc.vector.tensor_mul(junk, h[m], ab)
            zc = sb.tile([128, 1], F32, tag=f"zc{m}", name=f"zc{m}")
            nc.vector.tensor_reduce(zc, junk, mybir.AxisListType.X, OP.add)
            nc.sync.dma_start(out=zh_out.ap()[m * 128:(m + 1) * 128][:, None],
                              in_=zc)


# ---------------------------------------------------------------------------
_CACHE = {}


def _get_nc(debug=False):
    key = bool(debug)
    if key not in _CACHE:
        _CACHE[key] = build_nc(debug=debug)
    return _CACHE[key]


def _core_inputs(inputs, core):
    b, direc = core % Bb, core // Bb
    pre = "f" if direc == 0 else "b"
    x = np.asarray(inputs["x"][b], np.float32)
    if direc == 1:
        x = x[::-1]
    d = {"x_d": np.ascontiguousarray(x.T)}
    bf_names = {"inw", "xpw", "dtw", "ow", "w1", "w2"}
    for nm in ("inw", "cw", "cb", "xpw", "dtw", "dtb", "alog", "dd", "ow",
               "n1w", "n1b", "n2w", "n2b", "w1", "b1", "w2", "b2"):
        v = np.asarray(inputs[f"{pre}_{nm}"], np.float32)
        d[nm] = v.astype(BF) if nm in bf_names else v
    d["aw1"] = np.asarray(inputs["aw1"], np.float32).astype(BF)
    d["aw2"] = np.asarray(inputs["aw2"], np.float32).astype(BF)
    d["ab1"] = np.asarray(inputs["ab1"], np.float32)
    d["ab2"] = np.asarray(inputs["ab2"], np.float32)
    d["ones_colT"] = np.ones((128, 1), BF)
    d["ident"] = np.eye(128, dtype=np.float32).astype(BF)
    return d


def _host_ln(x, w, b):
    mu = x.mean(-1, keepdims=True)
    v = ((x - mu) ** 2).mean(-1, keepdims=True)
    return (x - mu) / np.sqrt(v + 1e-5) * w + b


def kernel(**inputs):
    res = run_cores(inputs)
    return assemble(inputs, res)


def run_cores(inputs, debug=False, trace=False):
    nc = _get_nc(debug=debug)
    in_maps = [_core_inputs(inputs, c) for c in range(8)]
    return bass_utils.run_bass_kernel_spmd(nc, in_maps, list(range(8)),
                                           trace=trace)


def assemble(inputs, res):
    z_cat = np.zeros((Bb, 2 * DM), np.float32)
    attn = np.zeros((Bb, N), np.float32)
    for b in range(Bb):
        zf = res.results[b]["zh"]
        zb = res.results[Bb + b]["zh"]
        af = res.results[b]["av"]
        ab = res.results[Bb + b]["av"][::-1]
        z_cat[b, :DM] = zf
        z_cat[b, DM:] = zb
        attn[b] = 0.5 * (af + ab)
    nw = np.asarray(inputs["nw"], np.float32)
    nb = np.asarray(inputs["nb"], np.float32)
    z = _host_ln(z_cat, nw, nb).astype(np.float32)
    return z, attn



# revision 2
# speedup vs baseline: 1.1255x; 1.1255x over previous
"""BiMamba aggregator v2 — engine-balanced rewrite.

Sharding: 8 shards = batch(4) x direction(2), one full sequence+stack per
core (same as baseline). Differences vs baseline:

- Selective scan: s-values fused in blocks of SBLK=2 into single
  tensor_tensor_scan ops [128, SBLK*1024] using a zero-decay boundary
  column (dt col 0 overwritten with -1e30 so exp(+A*dt) gives 0; the t=0
  decay multiplies h_init=0 and is never needed).
- The DS=16 reduction accumulates in ONE PSUM pair per m-tile across all
  16 s (no intermediate y tiles); dd*xhs is folded in as one extra
  diag(dd) matmul into the same PSUM; the reduction matmul uses -identity
  so the sign of u (= -dt*xh*B from the ln-sigmoid softplus trick)
  cancels.
- Depthwise conv runs on PE with host-built diagonal tap matrices.
- LN pipeline is bf16 end-to-end (stats via 1/DM-scaled ones matmul).
- u/v multiplies split between DVE (fused TT) and Pool (per-seg TT).
- m-tiles processed in two halves of 4 so the per-half B/C broadcast
  fits SBUF and the 8 PSUM banks hold all live py accumulators.
"""
import numpy as np
import ml_dtypes

import concourse.bass as bass
import concourse.tile as tile
from concourse import mybir
from concourse import bass_utils

F32 = mybir.dt.float32
BF16 = mybir.dt.bfloat16
AF = mybir.ActivationFunctionType
OP = mybir.AluOpType

DM, DI, DS, DC, DTR, L = 512, 1024, 16, 4, 32, 2
Bb, N = 4, 1024
NT2 = N // 2
SBLK = 2                   # s per fused scan
NBLK = DS // SBLK          # 8
SW = SBLK * N              # scan width
SLAB = DC - 1 + N          # padded width shared by xh / y3 slabs
BIGNEG = -1e30

BF = ml_dtypes.bfloat16


_EXEMPT = (
    mybir.InstEventSemaphore,
    mybir.InstAllEngineBarrier,
    mybir.InstHalt,
    mybir.InstCall,
)


def _legalize_waits(nc) -> int:
    n_split = 0
    for f in nc.m.functions:
        for bb in f.blocks:
            insts = bb.instructions
            if not any(
                (not isinstance(i, _EXEMPT))
                and i.sync_info is not None
                and len(i.sync_info.on_wait) > 1
                for i in insts
            ):
                continue
            new = []
            for i in insts:
                si = i.sync_info
                if isinstance(i, _EXEMPT) or si is None:
                    new.append(i)
                    continue
                waits = list(si.on_wait)
                if len(waits) <= 1:
                    new.append(i)
                    continue
                for w in waits[:-1]:
                    nop = mybir.InstNoOp(
                        name=f"{i.name}-wsplit{n_split}",
                        engine=i.engine,
                        sync_info=mybir.SyncInfo(on_wait=[w], on_update=[]),
                    )
                    new.append(nop)
                    n_split += 1
                i.sync_info = mybir.SyncInfo(
                    on_wait=waits[-1:], on_update=list(si.on_update)
                )
                new.append(i)
            bb.instructions = new
    return n_split


# ---------------------------------------------------------------------------
def build_nc(debug=False):
    nc = bass.Bass("TRN2", target_bir_lowering=False, debug=False)

    x_d = nc.dram_tensor("k2_x", [DM, N], F32, kind="ExternalInput")
    wt = {}

    def din(name, shape, dt):
        wt[name] = nc.dram_tensor("k2_" + name, shape, dt, kind="ExternalInput")

    din("inw", [L, DM, 2 * DI], BF16)
    din("diagw", [L, 8, 128, DC * 128], BF16)   # conv taps as diag blocks
    din("ddg", [L, 8, 128, 128], BF16)          # diag(dd)
    din("cb", [L, DI], F32)
    din("xpw", [L, DI, DTR + 2 * DS], BF16)
    din("dtw", [L, DTR, DI], BF16)
    din("dtb", [L, DI], F32)
    din("pA", [L, DI, DS], F32)                 # +exp(alog)
    din("ow", [L, DI, DM], BF16)
    din("n1w", [L, DM], F32)
    din("n1b", [L, DM], F32)
    din("n2w", [L, DM], F32)
    din("n2b", [L, DM], F32)
    din("w1", [L, DM, 4 * DM], BF16)
    din("b1", [L, 4 * DM], F32)
    din("w2", [L, 4 * DM, DM], BF16)
    din("b2", [L, DM], F32)
    din("aw1", [DM, DM // 2], BF16)
    din("ab1", [DM // 2], F32)
    din("aw2", [DM // 2, 1], BF16)
    din("ab2", [1], F32)
    din("onesDM", [128, 1], BF16)               # 1/DM column (LN stats)
    din("ident", [128, 128], BF16)

    zh_out = nc.dram_tensor("k2_zh", [DM], F32, kind="ExternalOutput")
    av_out = nc.dram_tensor("k2_av", [N], F32, kind="ExternalOutput")
    dbg = {}
    if debug:
        for nm, shape, dt in [
            ("d_xn0", [DM, N], BF16), ("d_xhs0", [DI, N], BF16),
            ("d_dt0", [DI, N], BF16), ("d_y30", [DI, N], BF16),
            ("d_h1", [DM, N], F32), ("d_h2", [DM, N], F32),
            ("d_hf", [DM, N], F32),
        ]:
            dbg[nm] = nc.dram_tensor("k2_" + nm, shape, dt,
                                     kind="ExternalOutput")

    with tile.TileContext(nc) as tc:
        _emit(nc, tc, x_d, wt, zh_out, av_out, dbg)

    _legalize_waits(nc)
    return nc


def _emit(nc, tc, x_d, wt, zh_out, av_out, dbg):
    import contextlib
    ctx = contextlib.ExitStack()
    with ctx:
        sb = ctx.enter_context(tc.tile_pool(name="sb", bufs=1))
        ps = ctx.enter_context(tc.tile_pool(name="ps", bufs=1, space="PSUM"))
        dr = ctx.enter_context(tc.tile_pool(name="dr", bufs=1, space="DRAM"))

        def st(shape, dt, tag, bufs):
            return sb.tile(shape, dt, tag=tag, bufs=bufs, name=tag)

        def pp(shape, name):
            return ps.tile(shape, F32, tag="pp", bufs=8, name=name)

        # ---- constants ----
        onesDM = sb.tile([128, 1], BF16, tag="conesDM", name="conesDM")
        nc.sync.dma_start(out=onesDM, in_=wt["onesDM"].ap())
        ident = sb.tile([128, 128], BF16, tag="cident", name="cident")
        nc.sync.dma_start(out=ident, in_=wt["ident"].ap())
        eps_t = sb.tile([1, 1], F32, tag="ceps", name="ceps")
        nc.vector.memset(eps_t, 1e-5)

        # ---- load x as h gen-0 (feature-major) ----
        h = []
        for m in range(4):
            t = st([128, N], F32, "h", 4)
            nc.sync.dma_start(out=t, in_=x_d.ap()[m * 128:(m + 1) * 128, :])
            h.append(t)

        _COLSPEC = {"cb": (8, 1), "dtb": (8, 1),
                    "n1w": (4, 1), "n1b": (4, 1), "n2w": (4, 1),
                    "n2b": (4, 1), "b1": (16, 1), "b2": (4, 1),
                    "pA": (8, DS)}
        cols = {}

        def col(name, l, m):
            cnt, width = _COLSPEC[name]
            key = (name, l)
            if key not in cols:
                t = sb.tile([128, cnt * width], F32, tag=f"{name}{l}",
                            name=f"{name}{l}")
                src = bass.AP(
                    tensor=wt[name], offset=l * cnt * 128 * width,
                    ap=[[width, 128], [128 * width, cnt], [1, width]])
                dst = t[:].rearrange("p (m k) -> p m k", k=width)
                nc.sync.dma_start(out=dst, in_=src)
                cols[key] = t
            t = cols[key]
            return t[:, m * width:(m + 1) * width]

        def rep(ap_t, times, width):
            """free-axis stride-0 repeat of a [128, width] tile AP."""
            a0 = ap_t[:]
            return bass.AP(tensor=a0.tensor, offset=a0.offset,
                           ap=[list(a0.ap[0]), [0, times], [1, width]])

        # ------------------------------------------------------------------
        def layernorm(l, h_tiles, wname, bname):
            hb, sq = [], []
            for m in range(4):
                hbm = st([128, N], BF16, "bfc", 5)
                nc.vector.tensor_scalar_mul(hbm, h_tiles[m], 1.0)
                hb.append(hbm)
                sqm = [st([128, NT2], BF16, "tmph", 3) for _ in range(2)]
                for n in range(2):
                    nc.vector.tensor_mul(
                        sqm[n], hbm[:, n * NT2:(n + 1) * NT2],
                        hbm[:, n * NT2:(n + 1) * NT2])
                sq.append(sqm)
            psmu = [pp([1, NT2], "psmu") for _ in range(2)]
            pssq = [pp([1, NT2], "pssq") for _ in range(2)]
            for m in range(4):
                for n in range(2):
                    sl = slice(n * NT2, (n + 1) * NT2)
                    nc.tensor.matmul(psmu[n], onesDM, hb[m][:, sl],
                                     start=(m == 0), stop=(m == 3))
                    nc.tensor.matmul(pssq[n], onesDM, sq[m][n],
                                     start=(m == 0), stop=(m == 3))
            lnsc = dr.tile([2, N], BF16, tag=f"lnsc_{l}_{wname}",
                           name=f"lnsc_{l}_{wname}")
            mb = st([128, N], BF16, "lnbc", 2)
            rb = st([128, N], BF16, "lnbc", 2)
            for n in range(2):
                sl = slice(n * NT2, (n + 1) * NT2)
                mu_row = sb.tile([1, NT2], BF16, tag="lnrow", bufs=4,
                                 name="murow")
                nc.vector.tensor_scalar_mul(mu_row, psmu[n], 1.0)
                musq = sb.tile([1, NT2], BF16, tag="lnrowh", bufs=2,
                               name="musq")
                nc.vector.tensor_mul(musq, mu_row, mu_row)
                var_row = sb.tile([1, NT2], BF16, tag="lnrow", bufs=4,
                                  name="varrow")
                nc.vector.tensor_sub(var_row, pssq[n], musq)
                sd_row = sb.tile([1, NT2], BF16, tag="lnrow", bufs=4,
                                 name="sdrow")
                nc.scalar.activation(sd_row, var_row, AF.Sqrt, bias=eps_t[:])
                rinv_row = sb.tile([1, NT2], BF16, tag="lnrow", bufs=4,
                                   name="rinvrow")
                with nc.allow_low_precision(reason="bf16 LN pipeline"):
                    nc.vector.reciprocal(rinv_row, sd_row)
                nc.sync.dma_start(out=lnsc[0:1, sl], in_=mu_row)
                nc.sync.dma_start(out=lnsc[1:2, sl], in_=rinv_row)
                nc.sync.dma_start(out=mb[:, sl], in_=bass.AP(
                    tensor=lnsc.tensor, offset=lnsc.offset + n * NT2,
                    ap=[[0, 128], [1, NT2]]))
                nc.sync.dma_start(out=rb[:, sl], in_=bass.AP(
                    tensor=lnsc.tensor, offset=lnsc.offset + N + n * NT2,
                    ap=[[0, 128], [1, NT2]]))
            outs = []
            for m in range(4):
                xo = st([128, N], BF16, "bfc", 5)
                for n in range(2):
                    sl = slice(n * NT2, (n + 1) * NT2)
                    s1 = st([128, NT2], BF16, "tmph", 3)
                    nc.vector.tensor_sub(s1, hb[m][:, sl], mb[:, sl])
                    s2 = st([128, NT2], BF16, "tmph", 3)
                    nc.vector.tensor_mul(s2, s1, rb[:, sl])
                    nc.vector.tensor_scalar(
                        out=xo[:, sl], in0=s2, scalar1=col(wname, l, m),
                        scalar2=col(bname, l, m), op0=OP.mult, op1=OP.add)
                outs.append(xo)
            return outs

        # =================== layers ===================
        for l in range(L):
            xn = layernorm(l, h, "n1w", "n1b")
            if dbg and l == 0:
                for m in range(4):
                    nc.sync.dma_start(
                        out=dbg["d_xn0"].ap()[m * 128:(m + 1) * 128, :],
                        in_=xn[m])

            # ---- in_proj ----
            inw_sb = []
            for j in range(4):
                t = st([128, 2 * DI], BF16, "w2048", 4)
                nc.sync.dma_start(
                    out=t, in_=wt["inw"].ap()[l, j * 128:(j + 1) * 128, :])
                inw_sb.append(t)
            xh = []
            for m in range(8):
                t = st([128, SLAB], BF16, "slab2k", 8)
                nc.vector.memset(t[:, 0:DC - 1], 0.0)
                xh.append(t)
            sz = [st([128, NT2], BF16, "sz", 16) for _ in range(16)]
            for mo in range(16):
                for n in range(2):
                    pm = pp([128, NT2], "pmm")
                    for j in range(4):
                        nc.tensor.matmul(
                            pm, inw_sb[j][:, mo * 128:(mo + 1) * 128],
                            xn[j][:, n * NT2:(n + 1) * NT2],
                            start=(j == 0), stop=(j == 3))
                    if mo < 8:
                        nc.scalar.copy(
                            xh[mo][:, DC - 1 + n * NT2:DC - 1 + (n + 1) * NT2],
                            pm)
                    else:
                        nc.scalar.activation(sz[(mo - 8) * 2 + n], pm, AF.Silu)

            # ---- conv on PE (diag taps) + silu ----
            xhs = []
            for m in range(8):
                dgw = st([128, DC * 128], BF16, "dgw", 2)
                nc.sync.dma_start(out=dgw, in_=wt["diagw"].ap()[l, m])
                t = st([128, N], BF16, "xhs", 8)
                for n in range(2):
                    pm = pp([128, NT2], "pcv")
                    for k in range(DC):
                        nc.tensor.matmul(
                            pm, dgw[:, k * 128:(k + 1) * 128],
                            xh[m][:, k + n * NT2:k + n * NT2 + NT2],
                            start=(k == 0), stop=(k == DC - 1))
                    nc.scalar.activation(t[:, n * NT2:(n + 1) * NT2], pm,
                                         AF.Silu, bias=col("cb", l, m))
                xhs.append(t)
            if dbg and l == 0:
                for m in range(8):
                    nc.sync.dma_start(
                        out=dbg["d_xhs0"].ap()[m * 128:(m + 1) * 128, :],
                        in_=xhs[m])

            # ---- x_proj -> dbl (B rows as-is, C rows negated) ----
            xpw_sb = []
            for j in range(8):
                t = st([128, DTR + 2 * DS], BF16, "xpw", 8)
                nc.sync.dma_start(
                    out=t, in_=wt["xpw"].ap()[l, j * 128:(j + 1) * 128, :])
                xpw_sb.append(t)
            dbl = sb.tile([64, N], BF16, tag="dblbf", bufs=1, name="dbl")
            for n in range(2):
                pm = pp([64, NT2], "pdbl")
                for j in range(8):
                    nc.tensor.matmul(pm, xpw_sb[j],
                                     xhs[j][:, n * NT2:(n + 1) * NT2],
                                     start=(j == 0), stop=(j == 7))
                nc.scalar.copy(dbl[:, n * NT2:(n + 1) * NT2], pm)

            bcsc = dr.tile([2 * DS, N], BF16, tag=f"bcsc{l}", name=f"bcsc{l}")
            nc.sync.dma_start(out=bcsc, in_=dbl[DTR:DTR + 2 * DS, :])

            dtw_sb = st([DTR, DI], BF16, "dtw", 1)
            nc.sync.dma_start(out=dtw_sb, in_=wt["dtw"].ap()[l])
            col("dtb", l, 0)
            ndtb = sb.tile([128, 8], F32, tag="ndtb", bufs=2, name=f"ndtb{l}")
            nc.vector.tensor_scalar_mul(ndtb, cols[("dtb", l)][:], -1.0)

            # ---- scan stage: two m-halves, B/C streamed per s-block ----
            y3 = [st([128, SLAB], BF16, "slab2k", 8) for _ in range(8)]
            for mh in range(2):
                ms = list(range(mh * 4, mh * 4 + 4))
                # dt (= -softplus via ln-sigmoid) + dtx for this half
                dt_bf, dtx = {}, {}
                for m in ms:
                    t = st([128, N], BF16, "dt", 4)
                    for n in range(2):
                        pm = pp([128, NT2], "pdt")
                        nc.tensor.matmul(pm,
                                         dtw_sb[:, m * 128:(m + 1) * 128],
                                         dbl[0:DTR, n * NT2:(n + 1) * NT2],
                                         start=True, stop=True)
                        sg = st([128, NT2], F32, "sg", 2)
                        nc.scalar.activation(sg, pm, AF.Sigmoid, scale=-1.0,
                                             bias=ndtb[:, m:m + 1])
                        nc.scalar.activation(t[:, n * NT2:(n + 1) * NT2], sg,
                                             AF.Ln)
                    dt_bf[m] = t
                    tx = st([128, N], BF16, "dtx", 4)
                    nc.vector.tensor_mul(tx, t, xhs[m])  # = -dt*xh
                    dtx[m] = tx
                    nc.gpsimd.memset(t[:, 0:1], BIGNEG)
                if dbg and l == 0:
                    for m in ms:
                        nc.sync.dma_start(
                            out=dbg["d_dt0"].ap()[m * 128:(m + 1) * 128, :],
                            in_=dt_bf[m])
                pys = {}
                for m in ms:
                    pys[m] = [pp([128, NT2], "py") for _ in range(2)]
                for blk in range(NBLK):
                    bt = st([128, SW], BF16, "BCb", 3)
                    nc.sync.dma_start(out=bt, in_=bass.AP(
                        tensor=bcsc.tensor,
                        offset=bcsc.offset + blk * SW,
                        ap=[[0, 128], [1, SW]]))
                    ct = st([128, SW], BF16, "BCb", 3)
                    nc.sync.dma_start(out=ct, in_=bass.AP(
                        tensor=bcsc.tensor,
                        offset=bcsc.offset + DS * N + blk * SW,
                        ap=[[0, 128], [1, SW]]))
                    for m in ms:
                        a4 = st([128, SW], BF16, "a4", 2)
                        for sl in range(SBLK):
                            s = blk * SBLK + sl
                            nc.scalar.activation(
                                a4[:, sl * N:(sl + 1) * N], dt_bf[m], AF.Exp,
                                scale=col("pA", l, m)[:, s:s + 1])
                        u4 = st([128, SW], BF16, "u4", 2)
                        u_dve = ((m * NBLK + blk) % 6 == 5)
                        if u_dve:
                            nc.vector.tensor_mul(u4, rep(dtx[m], SBLK, N), bt)
                        else:
                            for sl in range(SBLK):
                                nc.gpsimd.tensor_mul(
                                    u4[:, sl * N:(sl + 1) * N], dtx[m],
                                    bt[:, sl * N:(sl + 1) * N])
                        h4 = st([128, SW], BF16, "h4", 2)
                        nc.vector.tensor_tensor_scan(h4, a4, u4, 0.0,
                                                     OP.mult, OP.add)
                        v4 = st([128, SW], BF16, "v4", 1)
                        nc.vector.tensor_mul(v4, h4, ct)
                        for sl in range(SBLK):
                            for n in range(2):
                                nc.tensor.matmul(
                                    pys[m][n], ident,
                                    v4[:, sl * N + n * NT2:sl * N + (n + 1) * NT2],
                                    start=(blk == 0 and sl == 0), stop=False)
                for m in ms:
                    ddg = st([128, 128], BF16, "ddg", 2)
                    nc.sync.dma_start(out=ddg, in_=wt["ddg"].ap()[l, m])
                    for n in range(2):
                        nc.tensor.matmul(pys[m][n], ddg,
                                         xhs[m][:, n * NT2:(n + 1) * NT2],
                                         start=False, stop=True)
                        nc.vector.tensor_mul(
                            y3[m][:, n * NT2:(n + 1) * NT2], pys[m][n],
                            sz[m * 2 + n])
            if dbg and l == 0:
                for m in range(8):
                    nc.sync.dma_start(
                        out=dbg["d_y30"].ap()[m * 128:(m + 1) * 128, :],
                        in_=y3[m][:, 0:N])

            # ---- out_proj + residual ----
            ow_sb = []
            for j in range(8):
                t = st([128, DM], BF16, "w512", 16)
                nc.sync.dma_start(
                    out=t, in_=wt["ow"].ap()[l, j * 128:(j + 1) * 128, :])
                ow_sb.append(t)
            for mo in range(4):
                for n in range(2):
                    pm = pp([128, NT2], "pop")
                    for j in range(8):
                        nc.tensor.matmul(
                            pm, ow_sb[j][:, mo * 128:(mo + 1) * 128],
                            y3[j][:, n * NT2:(n + 1) * NT2],
                            start=(j == 0), stop=(j == 7))
                    nc.vector.tensor_add(
                        h[mo][:, n * NT2:(n + 1) * NT2],
                        h[mo][:, n * NT2:(n + 1) * NT2], pm)
            if dbg and l == 0:
                for m in range(4):
                    nc.sync.dma_start(
                        out=dbg["d_h1"].ap()[m * 128:(m + 1) * 128, :],
                        in_=h[m])

            # ---- LN2 + FFN ----
            hn = layernorm(l, h, "n2w", "n2b")
            w1_sb = []
            for j in range(4):
                t = st([128, 4 * DM], BF16, "w2048", 4)
                nc.sync.dma_start(
                    out=t, in_=wt["w1"].ap()[l, j * 128:(j + 1) * 128, :])
                w1_sb.append(t)
            w2_sb = []
            for j in range(16):
                t = st([128, DM], BF16, "w512", 16)
                nc.sync.dma_start(
                    out=t, in_=wt["w2"].ap()[l, j * 128:(j + 1) * 128, :])
                w2_sb.append(t)
            for n in range(2):
                gf = [st([128, NT2], BF16, "sz", 16) for _ in range(16)]
                for mo in range(16):
                    pm = pp([128, NT2], "pw1")
                    for j in range(4):
                        nc.tensor.matmul(
                            pm, w1_sb[j][:, mo * 128:(mo + 1) * 128],
                            hn[j][:, n * NT2:(n + 1) * NT2],
                            start=(j == 0), stop=(j == 3))
                    nc.scalar.activation(gf[mo], pm, AF.Gelu,
                                         bias=col("b1", l, mo))
                for mo in range(4):
                    pm = pp([128, NT2], "pw2")
                    for j in range(16):
                        nc.tensor.matmul(
                            pm, w2_sb[j][:, mo * 128:(mo + 1) * 128],
                            gf[j], start=(j == 0), stop=(j == 15))
                    nc.vector.scalar_tensor_tensor(
                        out=h[mo][:, n * NT2:(n + 1) * NT2], in0=pm,
                        scalar=col("b2", l, mo),
                        in1=h[mo][:, n * NT2:(n + 1) * NT2],
                        op0=OP.add, op1=OP.add)
            if dbg and l == 0:
                for m in range(4):
                    nc.sync.dma_start(
                        out=dbg["d_h2"].ap()[m * 128:(m + 1) * 128, :],
                        in_=h[m])

        # =================== attention pooling ===================
        if dbg:
            for m in range(4):
                nc.sync.dma_start(
                    out=dbg["d_hf"].ap()[m * 128:(m + 1) * 128, :], in_=h[m])
        aw1_sb = []
        for j in range(4):
            t = sb.tile([128, DM // 2], BF16, tag="aw1", bufs=4,
                        name=f"aw1_{j}")
            nc.sync.dma_start(out=t,
                              in_=wt["aw1"].ap()[j * 128:(j + 1) * 128, :])
            aw1_sb.append(t)
        ab1c = []
        for mg in range(2):
            t = sb.tile([128, 1], F32, tag=f"ab1_{mg}", name=f"ab1_{mg}")
            nc.sync.dma_start(
                out=t, in_=wt["ab1"].ap()[mg * 128:(mg + 1) * 128][:, None])
            ab1c.append(t)
        hbf = []
        for m in range(4):
            t = st([128, N], BF16, "bfc", 5)
            nc.vector.tensor_scalar_mul(t, h[m], 1.0)
            hbf.append(t)
        g1 = []
        for mg in range(2):
            t = st([128, N], BF16, "lnbc", 2)
            for n in range(2):
                pm = pp([128, NT2], "pg1")
                for j in range(4):
                    nc.tensor.matmul(
                        pm, aw1_sb[j][:, mg * 128:(mg + 1) * 128],
                        hbf[j][:, n * NT2:(n + 1) * NT2],
                        start=(j == 0), stop=(j == 3))
                nc.scalar.activation(t[:, n * NT2:(n + 1) * NT2], pm,
                                     AF.Tanh, bias=ab1c[mg])
            g1.append(t)
        aw2_sb = []
        for mg in range(2):
            t = sb.tile([128, 1], BF16, tag=f"aw2_{mg}", name=f"aw2_{mg}")
            nc.sync.dma_start(out=t,
                              in_=wt["aw2"].ap()[mg * 128:(mg + 1) * 128, :])
            aw2_sb.append(t)
        ab2_sb = sb.tile([1, 1], F32, tag="ab2", name="ab2")
        nc.sync.dma_start(out=ab2_sb, in_=wt["ab2"].ap()[None, :])
        lrow = sb.tile([1, N], F32, tag="lnrowf", bufs=2, name="lrow")
        for n in range(2):
            pm = pp([1, NT2], "pl")
            for mg in range(2):
                nc.tensor.matmul(pm, aw2_sb[mg],
                                 g1[mg][:, n * NT2:(n + 1) * NT2],
                                 start=(mg == 0), stop=(mg == 1))
            nc.vector.tensor_scalar_add(lrow[:, n * NT2:(n + 1) * NT2], pm,
                                        ab2_sb[:])
        mx = sb.tile([1, 1], F32, tag="tiny", bufs=4, name="mx")
        nc.vector.tensor_reduce(mx, lrow, mybir.AxisListType.X, OP.max)
        nmx = sb.tile([1, 1], F32, tag="tiny", bufs=4, name="nmx")
        nc.vector.tensor_scalar_mul(nmx, mx, -1.0)
        erow = sb.tile([1, N], F32, tag="lnrowf", bufs=2, name="erow")
        nc.scalar.activation(erow, lrow, AF.Exp, bias=nmx[:])
        ssum = sb.tile([1, 1], F32, tag="tiny", bufs=4, name="ssum")
        nc.vector.tensor_reduce(ssum, erow, mybir.AxisListType.X, OP.add)
        rs = sb.tile([1, 1], F32, tag="tiny", bufs=4, name="rs")
        nc.vector.reciprocal(rs, ssum)
        arow = sb.tile([1, N], F32, tag="lnrowf", bufs=2, name="arow")
        nc.vector.tensor_scalar_mul(arow, erow, rs[:])
        nc.sync.dma_start(out=av_out.ap()[None, :], in_=arow)
        arow_bf = sb.tile([1, N], BF16, tag="lnrow", bufs=4, name="arowbf")
        nc.vector.tensor_scalar_mul(arow_bf, arow, 1.0)
        absc = dr.tile([1, N], BF16, tag="absc", name="absc")
        nc.sync.dma_start(out=absc, in_=arow_bf)
        ab = st([128, N], BF16, "abbc", 1)
        nc.sync.dma_start(out=ab, in_=bass.AP(
            tensor=absc.tensor, offset=absc.offset, ap=[[0, 128], [1, N]]))
        for m in range(4):
            junk = st([128, N], BF16, "junk", 1)
            nc.vector.tensor_mul(junk, h[m], ab)
            zc = sb.tile([128, 1], F32, tag=f"zc{m}", name=f"zc{m}")
            nc.vector.tensor_reduce(zc, junk, mybir.AxisListType.X, OP.add)
            nc.sync.dma_start(out=zh_out.ap()[m * 128:(m + 1) * 128][:, None],
                              in_=zc)


# ---------------------------------------------------------------------------
_CACHE = {}


def _get_nc(debug=False):
    key = bool(debug)
    if key not in _CACHE:
        _CACHE[key] = build_nc(debug=debug)
    return _CACHE[key]


def _prep_weights(inputs, pre):
    """Host-side weight prep for one direction."""
    d = {}
    bf_names = {"inw", "xpw", "dtw", "ow", "w1", "w2"}
    for nm in ("inw", "cb", "xpw", "dtw", "dtb", "ow",
               "n1w", "n1b", "n2w", "n2b", "w1", "b1", "w2", "b2"):
        v = np.asarray(inputs[f"{pre}_{nm}"], np.float32)
        d[nm] = v.astype(BF) if nm in bf_names else v
    cw = np.asarray(inputs[f"{pre}_cw"], np.float32)       # [L, DI, DC]
    diagw = np.zeros((L, 8, 128, DC * 128), np.float32)
    idx = np.arange(128)
    for l in range(L):
        for m in range(8):
            for k in range(DC):
                diagw[l, m, idx, k * 128 + idx] = \
                    cw[l, m * 128:(m + 1) * 128, k]
    d["diagw"] = diagw.astype(BF)
    dd = np.asarray(inputs[f"{pre}_dd"], np.float32)       # [L, DI]
    ddg = np.zeros((L, 8, 128, 128), np.float32)
    for l in range(L):
        for m in range(8):
            ddg[l, m, idx, idx] = dd[l, m * 128:(m + 1) * 128]
    d["ddg"] = ddg.astype(BF)
    alog = np.asarray(inputs[f"{pre}_alog"], np.float32)
    d["pA"] = np.exp(alog).astype(np.float32)
    return d


def _core_inputs(inputs, core, shared):
    b, direc = core % Bb, core // Bb
    pre = "f" if direc == 0 else "b"
    x = np.asarray(inputs["x"][b], np.float32)
    if direc == 1:
        x = x[::-1]
    d = {"x": np.ascontiguousarray(x.T)}
    d.update(shared[pre])
    d["aw1"] = np.asarray(inputs["aw1"], np.float32).astype(BF)
    d["aw2"] = np.asarray(inputs["aw2"], np.float32).astype(BF)
    d["ab1"] = np.asarray(inputs["ab1"], np.float32)
    d["ab2"] = np.asarray(inputs["ab2"], np.float32)
    d["onesDM"] = np.full((128, 1), 1.0 / DM, BF)
    d["ident"] = (-np.eye(128, dtype=np.float32)).astype(BF)
    return {"k2_" + k: v for k, v in d.items()}


def _host_ln(x, w, b):
    mu = x.mean(-1, keepdims=True)
    v = ((x - mu) ** 2).mean(-1, keepdims=True)
    return (x - mu) / np.sqrt(v + 1e-5) * w + b


def kernel(**inputs):
    res = run_cores(inputs)
    return assemble(inputs, res)


def run_cores(inputs, debug=False, trace=False):
    nc = _get_nc(debug=debug)
    shared = {pre: _prep_weights(inputs, pre) for pre in ("f", "b")}
    in_maps = [_core_inputs(inputs, c, shared) for c in range(8)]
    return bass_utils.run_bass_kernel_spmd(nc, in_maps, list(range(8)),
                                           trace=trace)


def assemble(inputs, res):
    z_cat = np.zeros((Bb, 2 * DM), np.float32)
    attn = np.zeros((Bb, N), np.float32)
    for b in range(Bb):
        zf = res.results[b]["k2_zh"]
        zb = res.results[Bb + b]["k2_zh"]
        af = res.results[b]["k2_av"]
        abk = res.results[Bb + b]["k2_av"][::-1]
        z_cat[b, :DM] = zf
        z_cat[b, DM:] = zb
        attn[b] = 0.5 * (af + abk)
    nw = np.asarray(inputs["nw"], np.float32)
    nb = np.asarray(inputs["nb"], np.float32)
    z = _host_ln(z_cat, nw, nb).astype(np.float32)
    return z, attn
